# revision 53
# baseline (speedup 1.0000x reference)
"""AxialAttention2D kernel for 8 TRN2 NeuronCores.

Sharding: data-parallel over B (B == 8 == n_cores). Each core processes one
full [C, H, W] image: height pass (attend along W per row) + width pass
(attend along H per column), accumulating (xh + xw) / 2 + bias into an
SBUF-resident fp32 accumulator. No collectives.

v2b:
- SWDGE input DMA casts fp32->bf16 in flight (no on-chip cast pass).
- GPSIMD builds a transposed bf16 copy xwh[c,(w,h)] so the width pass
  reads contiguous matmul operands (strided moving operands were 3x PE).
- Flat single-buffered PSUM tiles (8 banks: s=4, qk=1 seq, va=1, rs=1,
  av/p=1) -- no rotating pool, so PE never queues behind slow evacs.
- Software-pipelined emission: iter i runs scores_i | q_{i+1} |
  rowsum/AV_{i-1} | k,v_{i+1} | proj_{i-1}; PE never waits on exp_i.
- Warm-up matmul burst to fire the HAM clock un-throttle.
- Evacuation ops alternate between Scalar/Vector engines to balance load.

Self-contained: shapes hardcoded (B=8, C=128, H=W=128, heads=4).
"""

import numpy as np
from contextlib import ExitStack

C = 128          # channels (= SBUF partitions)
L = 128          # attention sequence length (H or W)
HW = L * L       # flattened spatial size
HEADS = 4
HD = C // HEADS  # 32
SCALE = HD ** -0.5
SG = 4           # items per group
S_ITEMS = 48     # phase-0 H-items (covers input DMA + xwh chunk-0 build)
NCHUNK = 16
CHW = HW // NCHUNK  # 1024 columns per input chunk
WARMUP_MMS = 44  # junk matmuls at t=0 to fire the HAM un-throttle

_cache = {}

W_NAMES = ("wqT_h", "wkT_h", "wvT_h", "wpT_h", "wqT_w", "wkT_w", "wvT_w", "wpT_w")


def _build_nc():
    import concourse.bacc as bacc
    import concourse.tile as tile
    from concourse import mybir

    f32 = mybir.dt.float32
    bf16 = mybir.dt.bfloat16
    Exp = mybir.ActivationFunctionType.Exp
    Ident = mybir.ActivationFunctionType.Identity

    nc = bacc.Bacc(None, name="axial_attn")

    x_d = nc.dram_tensor("x", [C, HW], f32, kind="ExternalInput")
    w_d = {n: nc.dram_tensor(n, [C, C], bf16, kind="ExternalInput") for n in W_NAMES}
    bias_d = nc.dram_tensor("bias", [C, 1], f32, kind="ExternalInput")
    out_d = nc.dram_tensor("out", [C, HW], f32, kind="ExternalOutput")

    groups = [("h", g0, "init") for g0 in range(0, S_ITEMS, SG)]
    groups += [("w", g0, "w") for g0 in range(0, L, SG)]
    groups += [("h", g0, "add") for g0 in range(S_ITEMS, L, SG)]
    n = len(groups)

    with ExitStack() as ctx:
        tc = ctx.enter_context(tile.TileContext(nc))
        singles = ctx.enter_context(tc.tile_pool(name="singles", bufs=1))
        big = ctx.enter_context(tc.tile_pool(name="big", bufs=1))
        # bufs=3: SBUF is plentiful; extra slots remove cross-iteration
        # WAR edges (e.g. evac_{i+1} vs readers of the recycled slot)
        qkvp = ctx.enter_context(tc.tile_pool(name="qkvp", bufs=3))
        etp = ctx.enter_context(tc.tile_pool(name="etp", bufs=3))
        nrm = ctx.enter_context(tc.tile_pool(name="nrm", bufs=3))
        # PSUM (8 banks): s 4 | qk 1 (q,k sequential) | va 1 | rs 1 | av+p 1
        ps_s = ctx.enter_context(tc.tile_pool(name="ps_s", bufs=1, space="PSUM"))
        ps_qk = ctx.enter_context(tc.tile_pool(name="ps_qk", bufs=1, space="PSUM"))
        ps_va = ctx.enter_context(tc.tile_pool(name="ps_va", bufs=1, space="PSUM"))
        ps_rs = ctx.enter_context(tc.tile_pool(name="ps_rs", bufs=1, space="PSUM"))
        ps_avp = ctx.enter_context(tc.tile_pool(name="ps_avp", bufs=1, space="PSUM"))

        w_sb = {}
        for nm in W_NAMES:
            w_sb[nm] = singles.tile([C, C], bf16, tag=nm, name=nm)
            nc.sync.dma_start(out=w_sb[nm][:], in_=w_d[nm][:])
        bias_sb = singles.tile([C, 1], f32, tag="bias")
        nc.sync.dma_start(out=bias_sb[:], in_=bias_d[:])
        ones_sb = singles.tile([C, HD], bf16, tag="ones")
        nc.vector.memset(ones_sb[:], 1.0)

        xc = big.tile([C, HW], bf16, tag="x_hw")    # [c, (h w)]
        xwh = big.tile([C, HW], bf16, tag="x_wh")   # [c, (w h)]
        acc = big.tile([C, HW], f32, tag="acc")

        acc_v = acc[:].rearrange("c (h w) -> c w h", w=L)
        xc_v = xc[:].rearrange("c (h w) -> c w h", w=L)

        # input stream (SWDGE cast fp32->bf16), then gpsimd builds xwh in
        # w-major chunks: contiguous writes (fast), and chunk j unblocks
        # width-groups 2j/2j+1 progressively -- no transition stall
        for ci in range(NCHUNK):
            sl = slice(ci * CHW, (ci + 1) * CHW)
            nc.gpsimd.dma_start(out=xc[:, sl], in_=x_d[:, sl])
        for wj in range(NCHUNK):
            nc.gpsimd.tensor_copy(
                out=xwh[:, wj * CHW:(wj + 1) * CHW].rearrange(
                    "c (w h) -> c w h", h=L),
                in_=xc_v[:, wj * 8:(wj + 1) * 8, :])

        # HAM warm-up: junk matmuls on weight tiles into the s banks
        s_warm = ps_s.tile([C, 2048], f32, tag="s", name="s_warm")
        wlist = [w_sb[nm] for nm in W_NAMES]
        for r in range(WARMUP_MMS):
            dst = s_warm[:, (r % 16) * 128:(r % 16) * 128 + 128]
            nc.tensor.matmul(dst, wlist[r % 8][:], wlist[(r + 1) % 8][:],
                             start=True, stop=True)

        def xsrc(passc):
            return xc if passc == "h" else xwh

        qkv_sb = [None] * n   # [C,1536] bf16: q 0:512 | k 512:1024 | vT 1024:1536
        eT_t = [None] * n
        rr_t = [None] * n
        on_t = [None] * n

        def emit_q(j):
            passc, g0, _ = groups[j]
            x = xsrc(passc)
            qp = ps_qk.tile([C, 512], f32, tag="qk", name=f"q{j}")
            nc.tensor.matmul(qp[:], w_sb[f"wqT_{passc}"][:],
                             x[:, g0 * L:(g0 + SG) * L], start=True, stop=True)
            sb = qkvp.tile([C, 1536], bf16, tag="qkv_sb", name=f"qkv{j}")
            qkv_sb[j] = sb
            nc.vector.tensor_copy(out=sb[:, 0:512], in_=qp[:])

        def emit_k(j, keng):
            passc, g0, _ = groups[j]
            x = xsrc(passc)
            kp = ps_qk.tile([C, 512], f32, tag="qk", name=f"k{j}")
            nc.tensor.matmul(kp[:], w_sb[f"wkT_{passc}"][:],
                             x[:, g0 * L:(g0 + SG) * L], start=True, stop=True)
            sb = qkv_sb[j]
            if keng == "act":
                nc.scalar.copy(out=sb[:, 512:1024], in_=kp[:])
            else:
                nc.vector.tensor_copy(out=sb[:, 512:1024], in_=kp[:])

        def emit_v(j, veng):
            passc, g0, _ = groups[j]
            x = xsrc(passc)
            wv = w_sb[f"wvT_{passc}"]
            va = ps_va.tile([C, 512], f32, tag="va", name=f"va{j}")
            for it in range(SG):
                nc.tensor.matmul(va[:, it * L:(it + 1) * L],
                                 x[:, (g0 + it) * L:(g0 + it + 1) * L], wv[:],
                                 start=True, stop=True)
            sb = qkv_sb[j]
            if veng == "act":
                nc.scalar.copy(out=sb[:, 1024:1536], in_=va[:])
            else:
                nc.vector.tensor_copy(out=sb[:, 1024:1536], in_=va[:])

        def emit_scores_exp(i):
            sb = qkv_sb[i]
            s_ps = ps_s.tile([C, 2048], f32, tag="s", name=f"s{i}")
            for it in range(SG):
                for h in range(HEADS):
                    off = h * 512 + it * L
                    nc.tensor.matmul(
                        s_ps[:, off:off + L],
                        sb[HD * h:HD * h + HD, 512 + it * L:512 + (it + 1) * L],
                        sb[HD * h:HD * h + HD, it * L:(it + 1) * L],
                        start=True, stop=True, tile_position=(HD * h, 0))
            eT = etp.tile([C, 2048], bf16, tag="eT", name=f"eT{i}")
            eT_t[i] = eT
            nc.scalar.activation(out=eT[:], in_=s_ps[:], func=Exp, scale=SCALE)

        def emit_rs_recip(i):
            eT = eT_t[i]
            rs = ps_rs.tile([C, 512], f32, tag="rs", name=f"rs{i}")
            for h in range(HEADS):
                nc.tensor.matmul(rs[HD * h:HD * h + HD, :], ones_sb[:],
                                 eT[:, h * 512:(h + 1) * 512],
                                 start=True, stop=True, tile_position=(0, HD * h))
            rr = nrm.tile([C, 512], f32, tag="rr", name=f"rr{i}")
            rr_t[i] = rr
            nc.vector.reciprocal_approx_fast(out=rr[:], in_=rs[:])

        def emit_av_mul(i):
            eT = eT_t[i]
            sb = qkv_sb[i]
            av = ps_avp.tile([C, 512], f32, tag="avp", name=f"av{i}")
            for it in range(SG):
                for h in range(HEADS):
                    esl = eT[:, h * 512 + it * L:h * 512 + (it + 1) * L]
                    nc.tensor.matmul(
                        av[HD * h:HD * h + HD, it * L:(it + 1) * L],
                        sb[:, 1024 + it * L + HD * h:1024 + it * L + HD * h + HD],
                        esl, start=True, stop=True, tile_position=(0, HD * h))
            on = nrm.tile([C, 512], bf16, tag="on", name=f"on{i}")
            on_t[i] = on
            nc.vector.tensor_mul(out=on[:], in0=av[:], in1=rr_t[i][:])

        out_chunks = list(range(S_ITEMS * L // CHW))

        def emit_final(i):
            passc, g0, mode = groups[i]
            p = ps_avp.tile([C, 512], f32, tag="avp", name=f"p{i}")
            nc.tensor.matmul(p[:], w_sb[f"wpT_{passc}"][:], on_t[i][:],
                             start=True, stop=True)
            if mode == "init":
                nc.scalar.activation(out=acc[:, g0 * L:(g0 + SG) * L], in_=p[:],
                                     func=Ident, bias=bias_sb[:], scale=1.0)
            elif mode == "w":
                accv = acc_v[:, g0:g0 + SG, :]
                pv = p[:].rearrange("c (g l) -> c g l", g=SG)
                nc.vector.tensor_add(out=accv[:, :, 0:S_ITEMS],
                                     in0=pv[:, :, 0:S_ITEMS],
                                     in1=accv[:, :, 0:S_ITEMS])
                nc.scalar.activation(out=accv[:, :, S_ITEMS:L],
                                     in_=pv[:, :, S_ITEMS:L],
                                     func=Ident, bias=bias_sb[:], scale=1.0)
            else:  # "add"
                blk = acc[:, g0 * L:(g0 + SG) * L]
                nc.vector.tensor_add(out=blk, in0=p[:], in1=blk)
                nc.sync.dma_start(out=out_d[:, g0 * L:(g0 + SG) * L], in_=blk)
                if out_chunks:
                    ci = out_chunks.pop(0)
                    sl = slice(ci * CHW, (ci + 1) * CHW)
                    nc.sync.dma_start(out=out_d[:, sl], in_=acc[:, sl])

        # ---- software-pipelined schedule ----
        # iter i: scores_i+exp_i | q_{i+1} | rs/AV_{i-1} | k,v_{i+1} | proj_{i-1}
        emit_q(0)
        emit_k(0, "vec")
        emit_v(0, "act")
        for i in range(n + 1):
            if i < n:
                emit_scores_exp(i)
            if i + 1 < n:
                emit_q(i + 1)
            if 0 <= i - 1:
                emit_rs_recip(i - 1)
                emit_av_mul(i - 1)
            if i + 1 < n:
                # scalar is busy-bound in ph0/ph1 (it also carries the
                # init Identity there): push k-evac toward the DVE
                mode_next = groups[i + 1][2]
                if mode_next == "init":
                    emit_k(i + 1, "vec")
                elif mode_next == "w":
                    emit_k(i + 1, "vec" if i % 2 == 0 else "act")
                else:
                    emit_k(i + 1, "act")
                emit_v(i + 1, "vec" if i % 2 == 0 else "act")
            if 0 <= i - 1:
                emit_final(i - 1)

    nc.finalize()
    return nc


def _get_nc():
    if "nc" not in _cache:
        _cache["nc"] = _build_nc()
    return _cache["nc"]


def _make_in_maps(x, wqkv_h, wproj_h, bproj_h, wqkv_w, wproj_w, bproj_w):
    import ml_dtypes
    bf = ml_dtypes.bfloat16
    x = np.asarray(x, dtype=np.float32)
    B = x.shape[0]

    def wT(w):
        return np.ascontiguousarray(np.asarray(w, np.float32).T)

    common = {
        "wqT_h": wT(wqkv_h[0:C]).astype(bf),
        "wkT_h": wT(wqkv_h[C:2 * C]).astype(bf),
        "wvT_h": wT(wqkv_h[2 * C:3 * C]).astype(bf),
        "wpT_h": (wT(wproj_h) * 0.5).astype(bf),
        "wqT_w": wT(wqkv_w[0:C]).astype(bf),
        "wkT_w": wT(wqkv_w[C:2 * C]).astype(bf),
        "wvT_w": wT(wqkv_w[2 * C:3 * C]).astype(bf),
        "wpT_w": (wT(wproj_w) * 0.5).astype(bf),
        "bias": (0.5 * (np.asarray(bproj_h, np.float32)
                        + np.asarray(bproj_w, np.float32))).reshape(C, 1),
    }
    return [
        {**common, "x": np.ascontiguousarray(x[b].reshape(C, HW))}
        for b in range(B)
    ]


def _run(in_maps, **kw):
    from concourse.bass_utils import run_bass_kernel_spmd
    nc = _get_nc()
    res = run_bass_kernel_spmd(nc, in_maps, core_ids=list(range(len(in_maps))), **kw)
    _cache["last_results"] = res
    return res


def kernel(x, wqkv_h, wproj_h, bproj_h, wqkv_w, wproj_w, bproj_w):
    in_maps = _make_in_maps(x, wqkv_h, wproj_h, bproj_h,
                            wqkv_w, wproj_w, bproj_w)
    res = _run(in_maps)
    out = np.stack([r["out"].reshape(C, L, L) for r in res.results], axis=0)
    return out.astype(np.float32)


# revision 55
# speedup vs baseline: 1.0318x; 1.0318x over previous
"""AxialAttention2D kernel for 8 TRN2 NeuronCores.

Sharding: data-parallel over B (B == 8 == n_cores). Each core processes one
full [C, H, W] image: height pass (attend along W per row) + width pass
(attend along H per column), accumulating (xh + xw) / 2 + bias into an
SBUF-resident fp32 accumulator. No collectives.

v2b:
- SWDGE input DMA casts fp32->bf16 in flight (no on-chip cast pass).
- GPSIMD builds a transposed bf16 copy xwh[c,(w,h)] so the width pass
  reads contiguous matmul operands (strided moving operands were 3x PE).
- Flat single-buffered PSUM tiles (8 banks: s=4, qk=1 seq, va=1, rs=1,
  av/p=1) -- no rotating pool, so PE never queues behind slow evacs.
- Software-pipelined emission: iter i runs scores_i | q_{i+1} |
  rowsum/AV_{i-1} | k,v_{i+1} | proj_{i-1}; PE never waits on exp_i.
- Warm-up matmul burst to fire the HAM clock un-throttle.
- Evacuation ops alternate between Scalar/Vector engines to balance load.

Self-contained: shapes hardcoded (B=8, C=128, H=W=128, heads=4).
"""

import numpy as np
from contextlib import ExitStack

C = 128          # channels (= SBUF partitions)
L = 128          # attention sequence length (H or W)
HW = L * L       # flattened spatial size
HEADS = 4
HD = C // HEADS  # 32
SCALE = HD ** -0.5
SG = 4           # items per group
S_ITEMS = 48     # phase-0 H-items (covers input DMA + xwh chunk-0 build)
NCHUNK = 16
CHW = HW // NCHUNK  # 1024 columns per input chunk
WARMUP_MMS = 44  # junk matmuls at t=0 to fire the HAM un-throttle

_cache = {}

W_NAMES = ("wqT_h", "wkT_h", "wvT_h", "wpT_h", "wqT_w", "wkT_w", "wvT_w", "wpT_w")


def _build_nc():
    import concourse.bacc as bacc
    import concourse.tile as tile
    from concourse import mybir

    f32 = mybir.dt.float32
    bf16 = mybir.dt.bfloat16
    Exp = mybir.ActivationFunctionType.Exp
    Ident = mybir.ActivationFunctionType.Identity

    nc = bacc.Bacc(None, name="axial_attn")

    x_d = nc.dram_tensor("x", [C, HW], f32, kind="ExternalInput")
    w_d = {n: nc.dram_tensor(n, [C, C], bf16, kind="ExternalInput") for n in W_NAMES}
    bias_d = nc.dram_tensor("bias", [C, 1], f32, kind="ExternalInput")
    biasrow_d = nc.dram_tensor("biasrow", [C, 1024], f32, kind="ExternalInput")
    out_d = nc.dram_tensor("out", [C, HW], f32, kind="ExternalOutput")

    groups = [("h", g0, "init") for g0 in range(0, S_ITEMS, SG)]
    groups += [("w", g0, "w") for g0 in range(0, L, SG)]
    groups += [("h", g0, "add") for g0 in range(S_ITEMS, L, SG)]
    n = len(groups)

    with ExitStack() as ctx:
        tc = ctx.enter_context(tile.TileContext(nc))
        singles = ctx.enter_context(tc.tile_pool(name="singles", bufs=1))
        big = ctx.enter_context(tc.tile_pool(name="big", bufs=1))
        # bufs=3: SBUF is plentiful; extra slots remove cross-iteration
        # WAR edges (e.g. evac_{i+1} vs readers of the recycled slot)
        qkvp = ctx.enter_context(tc.tile_pool(name="qkvp", bufs=3))
        etp = ctx.enter_context(tc.tile_pool(name="etp", bufs=3))
        nrm = ctx.enter_context(tc.tile_pool(name="nrm", bufs=3))
        # PSUM (8 banks): s 4 | qk 1 (q,k sequential) | va 1 | rs 1 | av+p 1
        ps_s = ctx.enter_context(tc.tile_pool(name="ps_s", bufs=1, space="PSUM"))
        ps_qk = ctx.enter_context(tc.tile_pool(name="ps_qk", bufs=1, space="PSUM"))
        ps_va = ctx.enter_context(tc.tile_pool(name="ps_va", bufs=1, space="PSUM"))
        ps_rs = ctx.enter_context(tc.tile_pool(name="ps_rs", bufs=1, space="PSUM"))
        ps_avp = ctx.enter_context(tc.tile_pool(name="ps_avp", bufs=1, space="PSUM"))

        w_sb = {}
        for nm in W_NAMES:
            w_sb[nm] = singles.tile([C, C], bf16, tag=nm, name=nm)
            nc.sync.dma_start(out=w_sb[nm][:], in_=w_d[nm][:])
        bias_sb = singles.tile([C, 1], f32, tag="bias")
        nc.sync.dma_start(out=bias_sb[:], in_=bias_d[:])
        biasrow_sb = singles.tile([C, 1024], f32, tag="biasrow")
        nc.sync.dma_start(out=biasrow_sb[:], in_=biasrow_d[:])
        ones_sb = singles.tile([C, HD], bf16, tag="ones")
        nc.vector.memset(ones_sb[:], 1.0)

        xc = big.tile([C, HW], bf16, tag="x_hw")    # [c, (h w)]
        xwh = big.tile([C, HW], bf16, tag="x_wh")   # [c, (w h)]
        acc = big.tile([C, HW], f32, tag="acc")

        acc_v = acc[:].rearrange("c (h w) -> c w h", w=L)
        xc_v = xc[:].rearrange("c (h w) -> c w h", w=L)

        # input stream (SWDGE cast fp32->bf16), then gpsimd builds xwh in
        # w-major chunks: contiguous writes (fast), and chunk j unblocks
        # width-groups 2j/2j+1 progressively -- no transition stall
        for ci in range(NCHUNK):
            sl = slice(ci * CHW, (ci + 1) * CHW)
            nc.gpsimd.dma_start(out=xc[:, sl], in_=x_d[:, sl])
        for wj in range(NCHUNK):
            nc.gpsimd.tensor_copy(
                out=xwh[:, wj * CHW:(wj + 1) * CHW].rearrange(
                    "c (w h) -> c w h", h=L),
                in_=xc_v[:, wj * 8:(wj + 1) * 8, :])

        # HAM warm-up: junk matmuls on weight tiles into the s banks
        s_warm = ps_s.tile([C, 2048], f32, tag="s", name="s_warm")
        wlist = [w_sb[nm] for nm in W_NAMES]
        for r in range(WARMUP_MMS):
            dst = s_warm[:, (r % 16) * 128:(r % 16) * 128 + 128]
            nc.tensor.matmul(dst, wlist[r % 8][:], wlist[(r + 1) % 8][:],
                             start=True, stop=True)

        def xsrc(passc):
            return xc if passc == "h" else xwh

        qkv_sb = [None] * n   # [C,1536] bf16: q 0:512 | k 512:1024 | vT 1024:1536
        eT_t = [None] * n
        rr_t = [None] * n
        on_t = [None] * n

        def emit_q(j):
            passc, g0, _ = groups[j]
            x = xsrc(passc)
            qp = ps_qk.tile([C, 512], f32, tag="qk", name=f"q{j}")
            nc.tensor.matmul(qp[:], w_sb[f"wqT_{passc}"][:],
                             x[:, g0 * L:(g0 + SG) * L], start=True, stop=True)
            sb = qkvp.tile([C, 1536], bf16, tag="qkv_sb", name=f"qkv{j}")
            qkv_sb[j] = sb
            nc.vector.tensor_copy(out=sb[:, 0:512], in_=qp[:])

        def emit_k(j, keng):
            passc, g0, _ = groups[j]
            x = xsrc(passc)
            kp = ps_qk.tile([C, 512], f32, tag="qk", name=f"k{j}")
            nc.tensor.matmul(kp[:], w_sb[f"wkT_{passc}"][:],
                             x[:, g0 * L:(g0 + SG) * L], start=True, stop=True)
            sb = qkv_sb[j]
            if keng == "act":
                nc.scalar.copy(out=sb[:, 512:1024], in_=kp[:])
            else:
                nc.vector.tensor_copy(out=sb[:, 512:1024], in_=kp[:])

        def emit_v(j, veng):
            passc, g0, _ = groups[j]
            x = xsrc(passc)
            wv = w_sb[f"wvT_{passc}"]
            va = ps_va.tile([C, 512], f32, tag="va", name=f"va{j}")
            for it in range(SG):
                nc.tensor.matmul(va[:, it * L:(it + 1) * L],
                                 x[:, (g0 + it) * L:(g0 + it + 1) * L], wv[:],
                                 start=True, stop=True)
            sb = qkv_sb[j]
            if veng == "act":
                nc.scalar.copy(out=sb[:, 1024:1536], in_=va[:])
            else:
                nc.vector.tensor_copy(out=sb[:, 1024:1536], in_=va[:])

        def emit_scores_exp(i):
            sb = qkv_sb[i]
            s_ps = ps_s.tile([C, 2048], f32, tag="s", name=f"s{i}")
            for it in range(SG):
                for h in range(HEADS):
                    off = h * 512 + it * L
                    nc.tensor.matmul(
                        s_ps[:, off:off + L],
                        sb[HD * h:HD * h + HD, 512 + it * L:512 + (it + 1) * L],
                        sb[HD * h:HD * h + HD, it * L:(it + 1) * L],
                        start=True, stop=True, tile_position=(HD * h, 0))
            eT = etp.tile([C, 2048], bf16, tag="eT", name=f"eT{i}")
            eT_t[i] = eT
            nc.scalar.activation(out=eT[:], in_=s_ps[:], func=Exp, scale=SCALE)

        def emit_rs_recip(i):
            eT = eT_t[i]
            rs = ps_rs.tile([C, 512], f32, tag="rs", name=f"rs{i}")
            for h in range(HEADS):
                nc.tensor.matmul(rs[HD * h:HD * h + HD, :], ones_sb[:],
                                 eT[:, h * 512:(h + 1) * 512],
                                 start=True, stop=True, tile_position=(0, HD * h))
            rr = nrm.tile([C, 512], f32, tag="rr", name=f"rr{i}")
            rr_t[i] = rr
            nc.vector.reciprocal_approx_fast(out=rr[:], in_=rs[:])

        def emit_av_mul(i):
            eT = eT_t[i]
            sb = qkv_sb[i]
            av = ps_avp.tile([C, 512], f32, tag="avp", name=f"av{i}")
            for it in range(SG):
                for h in range(HEADS):
                    esl = eT[:, h * 512 + it * L:h * 512 + (it + 1) * L]
                    nc.tensor.matmul(
                        av[HD * h:HD * h + HD, it * L:(it + 1) * L],
                        sb[:, 1024 + it * L + HD * h:1024 + it * L + HD * h + HD],
                        esl, start=True, stop=True, tile_position=(0, HD * h))
            on = nrm.tile([C, 512], bf16, tag="on", name=f"on{i}")
            on_t[i] = on
            nc.vector.tensor_mul(out=on[:], in0=av[:], in1=rr_t[i][:])

        out_chunks = list(range(S_ITEMS * L // CHW))

        def emit_final(i):
            passc, g0, mode = groups[i]
            p = ps_avp.tile([C, 512], f32, tag="avp", name=f"p{i}")
            nc.tensor.matmul(p[:], w_sb[f"wpT_{passc}"][:], on_t[i][:],
                             start=True, stop=True)
            if mode == "init":
                nc.scalar.activation(out=acc[:, g0 * L:(g0 + SG) * L], in_=p[:],
                                     func=Ident, bias=bias_sb[:], scale=1.0)
            elif mode == "w":
                # rows [S_ITEMS:L] were pre-biased in phase-0, so one
                # full-width add with a CONTIGUOUS read of p replaces the
                # two strided-PSUM-read ops (add-part + Identity-part)
                accv = acc_v[:, g0:g0 + SG, :]
                pv = p[:].rearrange("c (g l) -> c g l", g=SG)
                nc.vector.tensor_add(out=accv, in0=pv, in1=accv)
            else:  # "add"
                blk = acc[:, g0 * L:(g0 + SG) * L]
                nc.vector.tensor_add(out=blk, in0=p[:], in1=blk)
                nc.sync.dma_start(out=out_d[:, g0 * L:(g0 + SG) * L], in_=blk)
                if out_chunks:
                    ci = out_chunks.pop(0)
                    sl = slice(ci * CHW, (ci + 1) * CHW)
                    nc.sync.dma_start(out=out_d[:, sl], in_=acc[:, sl])

        # ---- software-pipelined schedule ----
        # iter i: scores_i+exp_i | q_{i+1} | rs/AV_{i-1} | k,v_{i+1} | proj_{i-1}
        emit_q(0)
        emit_k(0, "vec")
        emit_v(0, "act")
        pre_chunks = list(range(S_ITEMS * L // CHW, NCHUNK))
        for i in range(n + 1):
            if i >= 1 and pre_chunks:
                cj = pre_chunks.pop(0)
                nc.vector.tensor_copy(out=acc[:, cj * CHW:(cj + 1) * CHW],
                                      in_=biasrow_sb[:])
            if i < n:
                emit_scores_exp(i)
            if i + 1 < n:
                emit_q(i + 1)
            if 0 <= i - 1:
                emit_rs_recip(i - 1)
                emit_av_mul(i - 1)
            if i + 1 < n:
                emit_k(i + 1, "act")
                emit_v(i + 1, "vec" if i % 2 == 0 else "act")
            if 0 <= i - 1:
                emit_final(i - 1)

    nc.finalize()
    return nc


def _get_nc():
    if "nc" not in _cache:
        _cache["nc"] = _build_nc()
    return _cache["nc"]


def _make_in_maps(x, wqkv_h, wproj_h, bproj_h, wqkv_w, wproj_w, bproj_w):
    import ml_dtypes
    bf = ml_dtypes.bfloat16
    x = np.asarray(x, dtype=np.float32)
    B = x.shape[0]

    def wT(w):
        return np.ascontiguousarray(np.asarray(w, np.float32).T)

    common = {
        "wqT_h": wT(wqkv_h[0:C]).astype(bf),
        "wkT_h": wT(wqkv_h[C:2 * C]).astype(bf),
        "wvT_h": wT(wqkv_h[2 * C:3 * C]).astype(bf),
        "wpT_h": (wT(wproj_h) * 0.5).astype(bf),
        "wqT_w": wT(wqkv_w[0:C]).astype(bf),
        "wkT_w": wT(wqkv_w[C:2 * C]).astype(bf),
        "wvT_w": wT(wqkv_w[2 * C:3 * C]).astype(bf),
        "wpT_w": (wT(wproj_w) * 0.5).astype(bf),
        "bias": (0.5 * (np.asarray(bproj_h, np.float32)
                        + np.asarray(bproj_w, np.float32))).reshape(C, 1),
    }
    common["biasrow"] = np.ascontiguousarray(
        np.broadcast_to(common["bias"], (C, 1024)))
    return [
        {**common, "x": np.ascontiguousarray(x[b].reshape(C, HW))}
        for b in range(B)
    ]


def _run(in_maps, **kw):
    from concourse.bass_utils import run_bass_kernel_spmd
    nc = _get_nc()
    res = run_bass_kernel_spmd(nc, in_maps, core_ids=list(range(len(in_maps))), **kw)
    _cache["last_results"] = res
    return res


def kernel(x, wqkv_h, wproj_h, bproj_h, wqkv_w, wproj_w, bproj_w):
    in_maps = _make_in_maps(x, wqkv_h, wproj_h, bproj_h,
                            wqkv_w, wproj_w, bproj_w)
    res = _run(in_maps)
    out = np.stack([r["out"].reshape(C, L, L) for r in res.results], axis=0)
    return out.astype(np.float32)
